# revision 11
# baseline (speedup 1.0000x reference)
"""MixGARCH Trainium2 kernel, v2: R=8 phase-decimated linear recurrence.

Reference: v_t = relu(bias + Wx @ o_t^2 + Wh * v_{t-1}) + 1e-6. All terms are
>= 0 so relu is identity => LINEAR recurrence  v_t = Wh*v_{t-1} + c_t with
c_t = b' + Wx @ o_t^2,  b' = bias + 1e-6.

Transforms used here:
 - Steady-state shift: vt = v_t - mu with mu = b'/(1-Wh) satisfies
   vt_t = Wh*vt_{t-1} + Wx @ o_t^2  (NO bias term).  Host adds mu back.
 - Scale: device works on 1024*vt (host packs xin = 32*series, squares give
   1024*o^2), keeping everything in fp16 normal range. Host divides by 1024.
 - R=8 decimation: the DVE scan runs on stride-8 groups only:
   vt[8m+7] = Wh^8 * vt[8(m-1)+7] + S[m],  S[m] = sum_q Wh^(7-q) c'[8m+q].
   The 7 intermediate phases are reconstructed as
   vt[8m+p] = Wh^(p+1)*vt[8(m-1)+7] + d_p[m], d_p[m] = sum_{q<=p} Wh^(p-q) c'.
   S and d_p come from PE matmuls over a phase-major packed x^2 layout;
   the Wh^(p+1)*v_prev term is added either by a diagonal matmul accumulated
   into the same PSUM (X streams) or by a DVE scalar_tensor_tensor (Y
   streams) after ACT copies PSUM->SBUF fp16.
 - 8 cores x 2 halves of 32768 steps; 256-step warmup decouples halves
   (Wh <= 0.9 => 0.9^256 ~ 2e-12). Core0/halfA starts exactly from vars0;
   its first 256 outputs are produced on the host (boundary tail) so the
   device uniformly emits groups [32, 4128) only.
"""

import os
import numpy as np

T = 524288
K = 64
NJ = 8
NCORES = 8
HALF = 32768
W = 256                 # warmup steps per half
TT = W + HALF           # 33024 scan steps per half
R = 8
G = TT // R             # 4128 groups
WG = W // R             # 32 warmup groups
F = 512                 # groups per tile
NT = (G - WG) // F      # 8 main tiles
GOUT = G - WG           # 4096 emitted groups per stream
VSCALE = 1024.0

N_DIAG = int(os.environ.get("MIXGARCH_NDIAG", "2"))   # X streams (PE diag)
N_POOL = int(os.environ.get("MIXGARCH_NPOOL", "1"))   # pool-add streams
N_STT = 7 - N_DIAG - N_POOL                            # DVE stt streams

_CACHE = {}


def _build_nc():
    import concourse.bacc as bacc
    import concourse.mybir as mybir
    import concourse.tile as tile

    f16 = mybir.dt.float16
    f32 = mybir.dt.float32
    MULT = mybir.AluOpType.mult
    ADD = mybir.AluOpType.add

    nc = bacc.Bacc(None, target_bir_lowering=False)
    xin = nc.dram_tensor("xin", [128, G], f16, kind="ExternalInput")
    wall = nc.dram_tensor("wall", [128, 8 * 128], f16, kind="ExternalInput")
    wdiag = nc.dram_tensor("wdiag", [128, N_DIAG * 128], f16, kind="ExternalInput")
    wh8 = nc.dram_tensor("wh8", [128, F], f16, kind="ExternalInput")
    whp = nc.dram_tensor("whp", [128, 8], f32, kind="ExternalInput")
    vinit = nc.dram_tensor("vinit", [128, 1], f32, kind="ExternalInput")
    vout = nc.dram_tensor("vout", [128, 8 * GOUT], f16, kind="ExternalOutput")

    NY = 7 - N_DIAG  # Y streams: p = N_DIAG .. 6

    with tile.TileContext(nc) as tc:
        with (
            tc.tile_pool(name="const", bufs=1) as cpool,
            tc.tile_pool(name="xb", bufs=1) as xpool,
            tc.tile_pool(name="stg", bufs=1) as spool,
            tc.tile_pool(name="tmp", bufs=2) as tpool,
            tc.tile_pool(name="psum", bufs=1, space="PSUM") as ps,
        ):
            wall_sb = cpool.tile([128, 8 * 128], f16)
            nc.gpsimd.dma_start(wall_sb[:], wall[:])
            wd_sb = cpool.tile([128, N_DIAG * 128], f16)
            nc.gpsimd.dma_start(wd_sb[:], wdiag[:])
            wh8_sb = cpool.tile([128, F], f16)
            nc.gpsimd.dma_start(wh8_sb[:], wh8[:])
            whp_sb = cpool.tile([128, 8], f32)
            nc.gpsimd.dma_start(whp_sb[:], whp[:])
            vi_sb = cpool.tile([128, 1], f32)
            nc.gpsimd.dma_start(vi_sb[:], vinit[:])

            xin_sb = xpool.tile([128, G], f16)
            x2 = xpool.tile([128, G], f16)
            # first chunk small so tile0+tile1 compute starts early
            bounds = [0, WG + F, WG + 3 * F, WG + 5 * F, WG + 7 * F, G]
            for q in range(len(bounds) - 1):
                sl = slice(bounds[q], bounds[q + 1])
                [nc.sync, nc.scalar][q % 2].dma_start(xin_sb[:, sl], xin[:, sl])
                nc.vector.tensor_tensor(x2[:, sl], xin_sb[:, sl], xin_sb[:, sl], MULT)

            scanst = spool.tile([128, 1 + G], f16)
            nc.vector.tensor_copy(scanst[:, 0:1], vi_sb[:])
            outst = spool.tile([128, 7, GOUT], f16)
            dstg = spool.tile([128, NY, GOUT], f16)

            psS = ps.tile([128, F], f32, tag="S")
            psX = None
            if N_DIAG:
                psX = ps.tile([128, N_DIAG * F], f32, tag="X")
            psY = ps.tile([128, NY * F], f32, tag="Y")

            def lhsT(p):
                return wall_sb[0:16 * (p + 1), 128 * p:128 * p + 128]

            def tile_range(k):
                # scan tile k: k=0 -> [0, WG); k>=1 -> [WG+F*(k-1), WG+F*k)
                if k == 0:
                    return 0, WG
                return WG + F * (k - 1), WG + F * k

            for k in range(NT + 2):  # k = 0..9
                if k <= NT:
                    a, b = tile_range(k)
                    n = b - a
                    # scan-input matmul (full 128-row prefix) and scan
                    nc.tensor.matmul(psS[:, 0:n], lhsT(7), x2[:, a:b],
                                     start=True, stop=True)
                    nc.vector.tensor_tensor_scan(
                        scanst[:, 1 + a:1 + b], wh8_sb[:, 0:n], psS[:, 0:n],
                        scanst[:, a:a + 1], MULT, ADD)

                if 1 <= k <= NT:
                    # Y streams: fill + ACT drain for this tile's window
                    a, b = tile_range(k)
                    n = b - a
                    for yi in range(NY):
                        p = N_DIAG + yi
                        nc.tensor.matmul(psY[:, F * yi:F * yi + n], lhsT(p),
                                         x2[0:16 * (p + 1), a:b],
                                         start=True, stop=True)
                    # merged PSUM->SBUF fp16 drain of all Y streams
                    nc.scalar.activation(
                        dstg[:, :, a - WG:b - WG], psY[:, 0:NY * F].rearrange(
                            "p (s f) -> p s f", s=NY)[:, :, 0:n],
                        mybir.ActivationFunctionType.Copy)

                if 2 <= k <= NT + 1:
                    # lagged emit round over window = tile (k-1)'s range
                    a, b = tile_range(k - 1)
                    n = b - a
                    vprev = scanst[:, a:b]
                    # X streams: d-matmul + diag accumulate + final ACT drain
                    for p in range(N_DIAG):
                        nc.tensor.matmul(psX[:, F * p:F * p + n], lhsT(p),
                                         x2[0:16 * (p + 1), a:b],
                                         start=True, stop=True)
                        nc.tensor.matmul(psX[:, F * p:F * p + n],
                                         wd_sb[:, 128 * p:128 * p + 128],
                                         vprev, start=False, stop=True)
                    if N_DIAG:
                        nc.scalar.activation(
                            outst[:, 0:N_DIAG, a - WG:b - WG],
                            psX[:, 0:N_DIAG * F].rearrange(
                                "p (s f) -> p s f", s=N_DIAG)[:, :, 0:n],
                            mybir.ActivationFunctionType.Copy)
                    # Y streams, already in dstg: scaled v_prev (ts, 4x mode)
                    # then tensor add (2x mode) / pool add
                    for yi in range(NY):
                        p = N_DIAG + yi
                        tmp = tpool.tile([128, F], f16, tag=f"vtmp{yi}")
                        nc.vector.tensor_scalar(
                            tmp[:, 0:n], vprev, whp_sb[:, p:p + 1], None,
                            MULT)
                        eng = nc.vector if p < 7 - N_POOL else nc.gpsimd
                        eng.tensor_tensor(
                            outst[:, p, a - WG:b - WG], tmp[:, 0:n],
                            dstg[:, yi, a - WG:b - WG], ADD)

                # output DMA waves: 1024-col chunks early, 512-col at the
                # tail so the final wave is small
                wave = {3: (0, 2 * F), 5: (2 * F, 4 * F), 7: (4 * F, 6 * F),
                        8: (6 * F, 7 * F), 9: (7 * F, 8 * F)}.get(k)
                if wave is not None:
                    c0, c1 = wave
                    engs = ([nc.sync, nc.gpsimd] if k < 9 else
                            [nc.sync, nc.gpsimd, nc.scalar])
                    for p in range(7):
                        engs[p % len(engs)].dma_start(
                            vout[:, GOUT * p + c0:GOUT * p + c1],
                            outst[:, p, c0:c1])
                    engs[0].dma_start(
                        vout[:, GOUT * 7 + c0:GOUT * 7 + c1],
                        scanst[:, 1 + WG + c0:1 + WG + c1])

    nc.compile()
    return nc


def _host_prep(series, vars0, bias, Wx, Wh):
    series = np.asarray(series, dtype=np.float32)
    vars0 = np.asarray(vars0, dtype=np.float32)
    bias = np.asarray(bias, dtype=np.float32)
    Wx = np.asarray(Wx, dtype=np.float32)
    Wh = np.asarray(Wh, dtype=np.float32)

    bprime = bias + 1e-6
    mu = bprime / (1.0 - Wh)

    # weights: wall[16q+8h+j, 128p+64h+k] = Wh_k^(p-q) * Wx[k,j], q<=p
    pows = np.ones((8, K), dtype=np.float32)  # pows[e] = Wh^e
    for e in range(1, 8):
        pows[e] = pows[e - 1] * Wh
    wall = np.zeros((128, 8 * 128), dtype=np.float32)
    for p in range(8):
        for q in range(p + 1):
            blk = pows[p - q][None, :] * Wx.T  # [j, k]
            for h in range(2):
                wall[16 * q + 8 * h:16 * q + 8 * h + 8,
                     128 * p + 64 * h:128 * p + 64 * h + 64] = blk
    wdiag = np.zeros((128, N_DIAG * 128), dtype=np.float32)
    for p in range(N_DIAG):
        coef = np.concatenate([pows[p] * Wh, pows[p] * Wh])  # Wh^(p+1), 128
        wdiag[np.arange(128), 128 * p + np.arange(128)] = coef
    wh8 = np.tile(np.concatenate([pows[7] * Wh, pows[7] * Wh])[:, None],
                  (1, F)).astype(np.float32)  # Wh^8 broadcast
    whp = np.zeros((128, 8), dtype=np.float32)
    for p in range(8):
        whp[0:64, p] = pows[p] * Wh if p < 7 else 0.0
        whp[64:128, p] = pows[p] * Wh if p < 7 else 0.0

    in_maps = []
    for c in range(NCORES):
        xinp = np.empty((128, G), dtype=np.float16)
        for h in range(2):
            if c == 0 and h == 0:
                t0 = 0
            else:
                t0 = c * 65536 + h * HALF - W
            rows = series[t0:t0 + TT, :]  # [TT, NJ]
            # xin[16q+8h+j, m] = 32*series[t0+8m+q, j]
            resh = (rows * 32.0).reshape(G, R, NJ)  # [m, q, j]
            blk = np.transpose(resh, (1, 2, 0))     # [q, j, m]
            xinp.reshape(R, 2, NJ, G)[:, h, :, :] = blk.astype(np.float16)
        vi = np.zeros((128, 1), dtype=np.float32)
        if c == 0:
            vi[0:64, 0] = (vars0 - mu) * VSCALE
        in_maps.append({
            "xin": xinp,
            "wall": wall.astype(np.float16),
            "wdiag": wdiag.astype(np.float16),
            "wh8": wh8.astype(np.float16),
            "whp": whp,
            "vinit": vi,
        })
    return in_maps, mu


def _assemble(results, mu, series, vars0, bias, Wx, Wh):
    hist = np.empty((T, K), dtype=np.float32)
    inv = np.float32(1.0 / VSCALE)
    for c in range(NCORES):
        v16 = results[c]["vout"]  # [128, 8*GOUT] fp16
        # [h, k, p, m] -> value/1024 + mu
        V = v16.astype(np.float32).reshape(2, K, 8, GOUT)
        for h in range(2):
            base = c * 65536 + h * HALF
            if c == 0 and h == 0:
                # device groups m>=WG map to t = 8*(m+WG)+p - ... here
                # stream col m corresponds to group (m+WG), t = 8*(m+WG)+p
                # valid t < 32768 -> group < 4096 -> col < 4096-WG
                ncols = GOUT - WG  # 4064
                arr = V[h, :, :, 0:ncols] * inv + mu[:, None, None]
                # hist[8*(col+WG)+p, k] ; t from 256 .. 32767
                hist[256:HALF, :] = np.transpose(
                    arr, (2, 1, 0)).reshape(ncols * 8, K)
            else:
                arr = V[h] * inv + mu[:, None, None]  # [k, p, GOUT]
                hist[base:base + HALF, :] = np.transpose(
                    arr, (2, 1, 0)).reshape(HALF, K)
    # exact boundary tail for core0/halfA: first 256 steps on host
    bprime = (np.asarray(bias, dtype=np.float64) + 1e-6)
    v = np.asarray(vars0, dtype=np.float64).copy()
    Wxd = np.asarray(Wx, dtype=np.float64)
    Whd = np.asarray(Wh, dtype=np.float64)
    s2 = np.asarray(series[0:W], dtype=np.float64) ** 2
    for t in range(W):
        v = bprime + Wxd @ s2[t] + Whd * v
        hist[t] = v.astype(np.float32)
    return hist


def run(inputs, trace=False, **kw):
    from concourse.bass_utils import run_bass_kernel_spmd

    if "nc" not in _CACHE:
        _CACHE["nc"] = _build_nc()
    nc = _CACHE["nc"]
    in_maps, mu = _host_prep(
        inputs["series"], inputs["vars0"], inputs["bias"],
        inputs["Wx"], inputs["Wh"],
    )
    res = run_bass_kernel_spmd(
        nc, in_maps, core_ids=list(range(NCORES)), trace=trace, **kw
    )
    out = _assemble(res.results, mu, inputs["series"], inputs["vars0"],
                    inputs["bias"], inputs["Wx"], inputs["Wh"])
    return out, res


def kernel(series, vars0, bias, Wx, Wh):
    out, _ = run(
        {"series": series, "vars0": vars0, "bias": bias, "Wx": Wx, "Wh": Wh}
    )
    return out
